# revision 39
# baseline (speedup 1.0000x reference)
"""AnomalyAttention TRN2 kernel: 8-core data-parallel Bass/Tile implementation.

Problem: B,C,H,L,E = 4,4,8,512,128. The 128 (b,c,h) slices are independent;
each of the 8 NeuronCores computes 16 slices. Per slice:
  scores = Q @ K^T (causal-masked), series = softmax(scores/sqrt(E)),
  V = series @ values, prior = gaussian(|i-j|; sigma(l)), sigma_out = bcast.

Self-contained: hardcodes shapes/sharding; host side only reshapes/
transposes (layout prep) and concatenates shard outputs.
"""

import math

import numpy as np

B, C, H, L, E = 4, 4, 8, 512, 128
P = 128
NB = L // P  # 4 row blocks of 128
N_CORES = 8
S = B * C * H  # 128 slices
NSLICE = S // N_CORES  # 16 per core

SCALE = 1.0 / math.sqrt(E)
LN3 = math.log(3.0)
SQRT2 = math.sqrt(2.0)
SQRT2PI = math.sqrt(2.0 * math.pi)
NEG = -1.0e30

_CACHE = {}


# prior band: for sig <= 3^(1+1e-5)-1 ~ 2.00007, exp(-d^2/(2 sig^2)) underflows
# to exactly 0.0 in f32 for |d| >= 65 (e.g. d=65 -> exp(-528)).  Each row block
# only needs a static column window covering |l-m| <= 64 for all its rows.
BAND_W = [192, 256, 256, 192]
BAND_M0 = [0, 64, 192, 320]
BAND_WMAX = 256


def _host_constants():
    idx = np.arange(L, dtype=np.float64)
    dist2 = (idx[:, None] - idx[None, :]) ** 2
    dist2negf = (-dist2).reshape(NB, P, L).astype(np.float32)
    # banded -dist^2 tiles: d2nband[i][p][c] = -((128 i + p) - (m0_i + c))^2
    d2nband = np.zeros((NB, P, BAND_WMAX), dtype=np.float32)
    for i in range(NB):
        rows = 128 * i + np.arange(P, dtype=np.float64)
        cols = BAND_M0[i] + np.arange(BAND_W[i], dtype=np.float64)
        d2nband[i, :, : BAND_W[i]] = -((rows[:, None] - cols[None, :]) ** 2)
    ii = np.arange(P)
    mask_lm = np.where(ii[None, :] > ii[:, None], NEG, 0.0).astype(np.float32)
    mask_ml = np.where(ii[None, :] < ii[:, None], NEG, 0.0).astype(np.float32)
    ident = np.eye(P, dtype=np.float32)
    # full-width causal mask rows, per row block: mask_lm_full[i][p, m] =
    # -1e30 where m > 128*i + p else 0 (only columns < w_i are ever used)
    ll = np.arange(L)
    mask_lm_full = np.zeros((NB, P, L), dtype=np.float32)
    for i in range(NB):
        rows = 128 * i + np.arange(P)
        mask_lm_full[i] = np.where(ll[None, :] > rows[:, None], NEG, 0.0)
    # scoresT diag mask extended with zeros to full width
    maskT_ext = np.zeros((P, L), dtype=np.float32)
    maskT_ext[:, :P] = mask_ml
    return {"dist2neg": d2nband, "dist2negf": dist2negf,
            "mask_lm": mask_lm, "mask_ml": mask_ml, "ident": ident,
            "mask_lm_full": mask_lm_full, "maskT_ext": maskT_ext}


def build(n_slices=NSLICE, n_cores=N_CORES, loop_iters=None, mode="full",
          split_series=True, bufs=2, sig_bcast=False, quad_outs=False,
          prior_ring="scalar", sigout_gpsimd=False, snorm_gpsimd=False,
          sigout_ring="sync", v_ring="sync", load_ring="sync",
          series_ring="sync",
          band_prior=True, m1=False, m2=False, m3=True, m4=True,
          ps_bufs=2, psT_bufs=2, series_bufs=None, drop_stores=False):
    """Build + compile the per-core SPMD graph. loop_iters wraps the body in a
    For_i loop (used only for wall-clock differential timing).

    mode: "full" | "dma" (loads + raw stores only, no compute).
    split_series: store only the nonzero prefix of each series row block
      (relies on output buffers being zero-initialized, which the PJRT
      donation path guarantees).
    sig_bcast: emit sigma_out as one broadcast-source DMA on the scalar
      HWDGE ring instead of materializing it in SBUF.
    """
    import concourse.tile as tile
    from concourse import bacc, mybir

    F32 = mybir.dt.float32
    Exp = mybir.ActivationFunctionType.Exp
    Square = mybir.ActivationFunctionType.Square
    Ln = mybir.ActivationFunctionType.Ln
    NS = n_slices

    nc = bacc.Bacc(
        "TRN2", target_bir_lowering=False, debug=False, num_devices=n_cores
    )

    qT = nc.dram_tensor("qT", [NS, E, L], F32, kind="ExternalInput").ap()
    kT = nc.dram_tensor("kT", [NS, E, L], F32, kind="ExternalInput").ap()
    vals = nc.dram_tensor("vals", [NS, L, E], F32, kind="ExternalInput").ap()
    sig_raw = nc.dram_tensor("sig_raw", [P, NS * NB], F32, kind="ExternalInput").ap()
    dist2neg = nc.dram_tensor(
        "dist2neg", [NB, P, BAND_WMAX], F32, kind="ExternalInput"
    ).ap()
    dist2negf = nc.dram_tensor(
        "dist2negf", [NB, P, L], F32, kind="ExternalInput"
    ).ap()
    BW = BAND_W if band_prior else [L] * NB
    BM0 = BAND_M0 if band_prior else [0] * NB
    BWMAX = BAND_WMAX if band_prior else L
    mask_lm = nc.dram_tensor("mask_lm", [P, P], F32, kind="ExternalInput").ap()
    mask_ml = nc.dram_tensor("mask_ml", [P, P], F32, kind="ExternalInput").ap()
    ident = nc.dram_tensor("ident", [P, P], F32, kind="ExternalInput").ap()
    mask_lm_full = nc.dram_tensor(
        "mask_lm_full", [NB, P, L], F32, kind="ExternalInput"
    ).ap()
    maskT_ext = nc.dram_tensor("maskT_ext", [P, L], F32, kind="ExternalInput").ap()

    v_out = nc.dram_tensor("v_out", [NS, L, E], F32, kind="ExternalOutput").ap()
    series_out = nc.dram_tensor(
        "series_out", [NS, L, L], F32, kind="ExternalOutput"
    ).ap()
    prior_out = nc.dram_tensor(
        "prior_out", [NS, L, L], F32, kind="ExternalOutput"
    ).ap()
    sigma_out = nc.dram_tensor(
        "sigma_out", [NS, L, L], F32, kind="ExternalOutput"
    ).ap()

    with tile.TileContext(nc) as tc:
        with (
            tc.tile_pool(name="const", bufs=1) as cpool,
            tc.tile_pool(name="sig", bufs=1) as sigpool,
            tc.tile_pool(name="qkv", bufs=3) as qkv_pool,
            tc.tile_pool(name="series", bufs=series_bufs or bufs) as spool,
            tc.tile_pool(name="seriesT", bufs=bufs) as stpool,
            tc.tile_pool(name="prior", bufs=bufs) as ppool,
            tc.tile_pool(name="sigout", bufs=bufs) as sopool,
            tc.tile_pool(name="vsb", bufs=bufs) as vpool,
            tc.tile_pool(name="small", bufs=bufs) as smallpool,
            tc.tile_pool(name="ps_s", bufs=ps_bufs, space="PSUM") as ps_pool,
            tc.tile_pool(name="ps_t", bufs=psT_bufs, space="PSUM") as psT_pool,
            tc.tile_pool(name="ps_v", bufs=2, space="PSUM") as psV_pool,
        ):
            # ---- constants ----
            d2n_t = cpool.tile([P, NB, BWMAX], F32)
            nc.sync.dma_start(
                d2n_t[:],
                (dist2neg if band_prior else dist2negf).rearrange(
                    "i p m -> p i m"
                ),
            )
            mlm_t = cpool.tile([P, P], F32)
            nc.sync.dma_start(mlm_t[:], mask_lm[:])
            mml_t = cpool.tile([P, P], F32)
            nc.sync.dma_start(mml_t[:], mask_ml[:])
            if m1:
                id_t = cpool.tile([P, P], F32)
                nc.sync.dma_start(id_t[:], ident[:])
                mlmf_t = cpool.tile([P, NB, L], F32)
                nc.sync.dma_start(
                    mlmf_t[:], mask_lm_full.rearrange("i p m -> p i m")
                )
                mmlx_t = cpool.tile([P, L], F32)
                nc.sync.dma_start(mmlx_t[:], maskT_ext[:])
            ones_t = cpool.tile([P, L], F32)
            nc.gpsimd.memset(ones_t[:], 1.0)

            # ---- sigma preprocessing (all slices at once) ----
            # sig = 3^(sigmoid(5x) + 1e-5) - 1 with high relative accuracy:
            # naive exp()-1 cancels for sigmoid ~ 0 and prior's peak is 1/sig.
            W = NS * NB
            sg_t = sigpool.tile([P, W], F32)
            nc.sync.dma_start(sg_t[:], sig_raw[:])
            sgm = sigpool.tile([P, W], F32)
            nc.scalar.activation(sgm[:], sg_t[:], Exp, scale=-5.0)
            nc.vector.tensor_scalar_add(sgm[:], sgm[:], 1.0)
            nc.vector.reciprocal(sgm[:], sgm[:])
            u_t = sigpool.tile([P, W], F32)
            nc.vector.tensor_scalar_mul(u_t[:], sgm[:], LN3)
            big = sigpool.tile([P, W], F32)
            nc.scalar.activation(big[:], u_t[:], Exp)
            nc.vector.tensor_scalar_add(big[:], big[:], -1.0)
            poly = sigpool.tile([P, W], F32)
            coef = [
                1.0 / 5040, 1.0 / 720, 1.0 / 120, 1.0 / 24,
                1.0 / 6, 1.0 / 2, 1.0,
            ]
            nc.vector.tensor_scalar(
                poly[:], u_t[:], coef[0], coef[1],
                op0=mybir.AluOpType.mult, op1=mybir.AluOpType.add,
            )
            for cc in coef[2:]:
                nc.vector.tensor_mul(poly[:], poly[:], u_t[:])
                nc.vector.tensor_scalar_add(poly[:], poly[:], cc)
            small = sigpool.tile([P, W], F32)
            nc.vector.tensor_mul(small[:], poly[:], u_t[:])
            selm = sigpool.tile([P, W], F32)
            nc.vector.tensor_scalar(
                selm[:], u_t[:], 0.5, None, op0=mybir.AluOpType.is_ge
            )
            nc.vector.tensor_sub(big[:], big[:], small[:])
            nc.vector.tensor_mul(big[:], big[:], selm[:])
            em1 = sigpool.tile([P, W], F32)
            nc.vector.tensor_add(em1[:], small[:], big[:])
            # t3y = 3^eps*(expm1(u)+1) with the constant split hi/lo so the
            # single big rounding mimics the reference's f32 pow(3, y).
            Cq = 3.0**1e-5
            C_hi = float(np.float32(Cq))
            C_lo = float(np.float32(Cq - C_hi))
            sig_t = sigpool.tile([P, W], F32)
            nc.vector.tensor_scalar(
                sig_t[:], em1[:], C_hi, C_lo,
                op0=mybir.AluOpType.mult, op1=mybir.AluOpType.add,
            )
            nc.vector.tensor_scalar_add(sig_t[:], sig_t[:], C_hi)
            nc.vector.tensor_scalar_add(sig_t[:], sig_t[:], -1.0)
            twosig2 = sigpool.tile([P, W], F32)
            nc.scalar.activation(twosig2[:], sig_t[:], Square, scale=SQRT2)
            inv2sig2 = sigpool.tile([P, W], F32)
            nc.vector.reciprocal(inv2sig2[:], twosig2[:])
            lnnorm = sigpool.tile([P, W], F32)
            nc.scalar.activation(lnnorm[:], sig_t[:], Ln, scale=SQRT2PI)
            nc.vector.tensor_scalar_mul(lnnorm[:], lnnorm[:], -1.0)

            def dma_only_body(iv=None):
                for s in range(NS):
                    qt = qkv_pool.tile([P, L], F32, tag="qt")
                    nc.sync.dma_start(qt[:], qT[s])
                    kt = qkv_pool.tile([P, L], F32, tag="kt")
                    nc.sync.dma_start(kt[:], kT[s])
                    vt = qkv_pool.tile([P, NB, E], F32, tag="vt")
                    nc.sync.dma_start(
                        vt[:], vals[s].rearrange("(j p) e -> p j e", p=P)
                    )
                    series_t = spool.tile([P, NB, L], F32)
                    prior_t = ppool.tile([P, NB, L], F32)
                    sigout_t = sopool.tile([P, NB, L], F32)
                    v_sb = vpool.tile([P, NB, E], F32)
                    nc.vector.tensor_copy(v_sb[:, 0, :], qt[:, :E])
                    nc.vector.tensor_copy(series_t[:, 0, :E], kt[:, :E])
                    nc.vector.tensor_copy(prior_t[:, 0, :E], kt[:, :E])
                    nc.vector.tensor_copy(sigout_t[:, 0, :E], qt[:, :E])
                    nc.sync.dma_start(
                        v_out[s].rearrange("(i p) e -> p i e", p=P), v_sb[:]
                    )
                    nc.sync.dma_start(
                        series_out[s].rearrange("(i p) m -> p i m", p=P),
                        series_t[:],
                    )
                    nc.sync.dma_start(
                        prior_out[s].rearrange("(i p) m -> p i m", p=P),
                        prior_t[:],
                    )
                    nc.sync.dma_start(
                        sigma_out[s].rearrange("(i p) m -> p i m", p=P),
                        sigout_t[:],
                    )

            def dma2_body(iv=None):
                # DMA probe: same bytes, 2-slice-batched transfers
                for s in range(0, NS, 2):
                    qt = qkv_pool.tile([P, 2, L], F32, tag="qt")
                    nc.sync.dma_start(qt[:], qT[s : s + 2].rearrange("s e l -> e s l"))
                    kt = qkv_pool.tile([P, 2, L], F32, tag="kt")
                    nc.sync.dma_start(kt[:], kT[s : s + 2].rearrange("s e l -> e s l"))
                    vt = qkv_pool.tile([P, 2 * NB, E], F32, tag="vt")
                    nc.sync.dma_start(
                        vt[:],
                        vals[s : s + 2].rearrange("s (j p) e -> p (s j) e", p=P),
                    )
                    series_t = spool.tile([P, 2 * NB, L], F32)
                    prior_t = ppool.tile([P, 2 * NB, L], F32)
                    sigout_t = sopool.tile([P, 2 * NB, L], F32)
                    v_sb = vpool.tile([P, 2 * NB, E], F32)
                    nc.vector.tensor_copy(v_sb[:, 0, :], qt[:, 0, :E])
                    nc.vector.tensor_copy(series_t[:, 0, :E], kt[:, 0, :E])
                    nc.vector.tensor_copy(prior_t[:, 0, :E], kt[:, 0, :E])
                    nc.vector.tensor_copy(sigout_t[:, 0, :E], qt[:, 0, :E])
                    nc.sync.dma_start(
                        v_out[s : s + 2].rearrange("s (i p) e -> p (s i) e", p=P),
                        v_sb[:],
                    )
                    nc.sync.dma_start(
                        series_out[s : s + 2].rearrange("s (i p) m -> p (s i) m", p=P),
                        series_t[:],
                    )
                    nc.sync.dma_start(
                        prior_out[s : s + 2].rearrange("s (i p) m -> p (s i) m", p=P),
                        prior_t[:],
                    )
                    nc.sync.dma_start(
                        sigma_out[s : s + 2].rearrange("s (i p) m -> p (s i) m", p=P),
                        sigout_t[:],
                    )

            def body(iv=None):
                ring = {"sync": nc.sync, "scalar": nc.scalar, "gpsimd": nc.gpsimd}
                prior_ce = ring[prior_ring]
                sigout_ce = ring[sigout_ring]
                series_ce = ring[series_ring]
                v_ce = ring[v_ring]
                load_ce = ring[load_ring]
                if sig_bcast:
                    # sigma_out = per-row broadcast of sig: one DMA for the
                    # whole tensor, broadcast (step-0) source, scalar ring.
                    # Depends only on the preamble, so it streams in the
                    # background for the entire kernel.
                    src = sig_t[:].rearrange("p (s i) -> p s i", s=NS)
                    srcb = src[:, :, :, None].to_broadcast([P, NS, NB, L])
                    dst = sigma_out.rearrange("s (i p) m -> p s i m", p=P)
                    nc.gpsimd.dma_start(dst, srcb)
                # process slices in pairs: all big DMA transfers are 2-slice
                # batched (small transfers are descriptor-overhead bound)
                prior_q = None
                v_q = None
                QB = min(4 if quad_outs else 2, NS)
                for sp in range(0, NS, 2):
                    qt = qkv_pool.tile([P, 2, L], F32, tag="qt")
                    load_ce.dma_start(
                        qt[:], qT[sp : sp + 2].rearrange("s e l -> e s l")
                    )
                    kt = qkv_pool.tile([P, 2, L], F32, tag="kt")
                    load_ce.dma_start(
                        kt[:], kT[sp : sp + 2].rearrange("s e l -> e s l")
                    )
                    vt = qkv_pool.tile([P, 2, NB, E], F32, tag="vt")
                    load_ce.dma_start(
                        vt[:],
                        vals[sp : sp + 2].rearrange("s (j p) e -> p s j e", p=P),
                    )

                    series_t = spool.tile([P, 2, NB, L], F32)
                    if sp % QB == 0:
                        prior_q = ppool.tile([P, QB, NB, BWMAX], F32)
                        v_q = vpool.tile([P, QB, NB, E], F32)
                    if not sig_bcast:
                        sigout_t = sopool.tile([P, 2, NB, L], F32)
                        if m2:
                            sb = (
                                sig_t[:, NB * sp : NB * (sp + 2)]
                                .rearrange("p (t i) -> p t i", t=2)[:, :, :, None]
                                .to_broadcast([P, 2, NB, L])
                            )
                            nc.vector.tensor_copy(sigout_t[:], sb)
                        else:
                            for tt_ in range(2):
                                for i in range(NB):
                                    cix = (sp + tt_) * NB + i
                                    nc.vector.tensor_scalar_mul(
                                        sigout_t[:, tt_, i, :],
                                        ones_t[:],
                                        sig_t[:, cix : cix + 1],
                                    )
                    qoff = sp % QB
                    rowsum = smallpool.tile([P, 2, NB], F32, tag="rowsum")
                    invsum = smallpool.tile([P, 2, NB], F32, tag="invsum")

                    for t in range(2):
                        s = sp + t
                        seriesT_t = stpool.tile([P, NB, L], F32)

                        # zero the masked tails (gpsimd is idle): keeps the
                        # merged normalize read fully initialized
                        for i in range(NB - 1):
                            w = P * (i + 1)
                            nc.gpsimd.memset(series_t[:, t, i, w:], 0.0)

                        # scores [l, m] blocks -> exp (+fused row-sum)
                        for i in range(NB):
                            w = P * (i + 1)
                            ps = ps_pool.tile([P, L], F32)
                            if m1:
                                # causal mask: seed -1e30 upper-tri via PE,
                                # then accumulate the scores on top
                                nc.tensor.matmul(
                                    ps[:, :w],
                                    id_t[:],
                                    mlmf_t[:, i, :w],
                                    start=True,
                                    stop=False,
                                    skip_group_check=True,
                                )
                                nc.tensor.matmul(
                                    ps[:, :w],
                                    qt[:, t, i * P : (i + 1) * P],
                                    kt[:, t, :w],
                                    start=False,
                                    stop=True,
                                    skip_group_check=True,
                                )
                            else:
                                nc.tensor.matmul(
                                    ps[:, :w],
                                    qt[:, t, i * P : (i + 1) * P],
                                    kt[:, t, :w],
                                    start=True,
                                    stop=True,
                                )
                                nc.vector.tensor_add(
                                    ps[:, i * P : w], ps[:, i * P : w], mlm_t[:]
                                )
                            nc.scalar.activation(
                                series_t[:, t, i, :w],
                                ps[:, :w],
                                Exp,
                                scale=SCALE,
                                accum_out=rowsum[:, t, i : i + 1],
                            )

                        # scores^T [m, l] blocks -> exp (feeds P@V)
                        for j in range(NB):
                            n = L - P * j
                            psT = psT_pool.tile([P, L], F32)
                            if m1:
                                nc.tensor.matmul(
                                    psT[:, :n],
                                    id_t[:],
                                    mmlx_t[:, :n],
                                    start=True,
                                    stop=False,
                                    skip_group_check=True,
                                )
                                nc.tensor.matmul(
                                    psT[:, :n],
                                    kt[:, t, j * P : (j + 1) * P],
                                    qt[:, t, j * P :],
                                    start=False,
                                    stop=True,
                                    skip_group_check=True,
                                )
                            else:
                                nc.tensor.matmul(
                                    psT[:, :n],
                                    kt[:, t, j * P : (j + 1) * P],
                                    qt[:, t, j * P :],
                                    start=True,
                                    stop=True,
                                )
                                nc.vector.tensor_add(
                                    psT[:, :P], psT[:, :P], mml_t[:]
                                )
                            nc.scalar.activation(
                                seriesT_t[:, j, :n], psT[:, :n], Exp, scale=SCALE
                            )

                        nc.vector.reciprocal(invsum[:, t, :], rowsum[:, t, :])
                        if m3:
                            inv_b = invsum[:, t, :, None].to_broadcast([P, NB, L])
                            nc.vector.tensor_mul(
                                series_t[:, t], series_t[:, t], inv_b
                            )
                        else:
                            for i in range(NB):
                                w = P * (i + 1)
                                nc.vector.tensor_scalar_mul(
                                    series_t[:, t, i, :w],
                                    series_t[:, t, i, :w],
                                    invsum[:, t, i : i + 1],
                                )

                        # V = softmax @ values (normalize on the evict)
                        if m4:
                            vps = psV_pool.tile([P, NB, E], F32)
                            for i in range(NB):
                                for j in range(i + 1):
                                    off = P * (i - j)
                                    nc.tensor.matmul(
                                        vps[:, i, :],
                                        seriesT_t[:, j, off : off + P],
                                        vt[:, t, j, :],
                                        start=(j == 0),
                                        stop=(j == i),
                                        skip_group_check=True,
                                    )
                            inv_e = invsum[:, t, :, None].to_broadcast(
                                [P, NB, E]
                            )
                            nc.vector.tensor_mul(
                                v_q[:, qoff + t], vps[:], inv_e
                            )
                        else:
                            for i in range(NB):
                                vps = psV_pool.tile([P, E], F32, tag="vps_s")
                                for j in range(i + 1):
                                    off = P * (i - j)
                                    nc.tensor.matmul(
                                        vps[:],
                                        seriesT_t[:, j, off : off + P],
                                        vt[:, t, j, :],
                                        start=(j == 0),
                                        stop=(j == i),
                                    )
                                nc.vector.tensor_scalar_mul(
                                    v_q[:, qoff + t, i, :],
                                    vps[:],
                                    invsum[:, t, i : i + 1],
                                )

                        # prior (banded) + sigma broadcast
                        for i in range(NB):
                            cix = s * NB + i
                            wb = BW[i]
                            nc.scalar.activation(
                                prior_q[:, qoff + t, i, :wb],
                                d2n_t[:, i, :wb],
                                Exp,
                                scale=inv2sig2[:, cix : cix + 1],
                                bias=lnnorm[:, cix : cix + 1],
                            )


                    # ---- batched stores ----
                    if drop_stores:
                        continue
                    sview = series_out[sp : sp + 2].rearrange(
                        "s (i p) m -> p s i m", p=P
                    )
                    if split_series:
                        # upper-triangle tail is all-zero; output buffers are
                        # zero-initialized, so store only the live prefix
                        for i in range(NB):
                            w = P * (i + 1)
                            series_ce.dma_start(
                                sview[:, :, i, :w], series_t[:, :, i, :w]
                            )
                    else:
                        nc.sync.dma_start(sview, series_t[:])
                    if not sig_bcast:
                        sigout_ce.dma_start(
                            sigma_out[sp : sp + 2].rearrange(
                                "s (i p) m -> p s i m", p=P
                            ),
                            sigout_t[:],
                        )
                    if qoff + 2 == QB:
                        s0 = sp + 2 - QB
                        v_ce.dma_start(
                            v_out[s0 : s0 + QB].rearrange(
                                "s (i p) e -> p s i e", p=P
                            ),
                            v_q[:],
                        )
                        pview = prior_out[s0 : s0 + QB].rearrange(
                            "s (i p) m -> p s i m", p=P
                        )
                        for i in range(NB):
                            wb = BW[i]
                            m0 = BM0[i]
                            prior_ce.dma_start(
                                pview[:, :, i, m0 : m0 + wb],
                                prior_q[:, :, i, :wb],
                            )

            the_body = {"dma": dma_only_body, "dma2": dma2_body}.get(mode, body)
            if loop_iters is not None and loop_iters > 1:
                with tc.For_i(0, loop_iters, 1) as iv:
                    the_body(iv)
            else:
                the_body()

    nc.compile()
    return nc


def _get_nc():
    if "nc" not in _CACHE:
        _CACHE["nc"] = build()
    return _CACHE["nc"]


def prep_core_inputs(q, k, v, sg):
    """Host-side layout prep for one core's shard (q,k,v: [n,L,E]; sg: [n,L])."""
    n = q.shape[0]
    out = {
        "qT": np.ascontiguousarray(np.swapaxes(q, 1, 2)),
        "kT": np.ascontiguousarray(np.swapaxes(k, 1, 2)),
        "vals": np.ascontiguousarray(v),
        "sig_raw": np.ascontiguousarray(
            sg.reshape(n, NB, P).transpose(2, 0, 1).reshape(P, n * NB)
        ),
    }
    out.update(_host_constants())
    return out


def kernel(queries, keys, values, sigma, attn_mask):
    from concourse import bass_utils

    nc = _get_nc()

    q = np.asarray(queries, dtype=np.float32).reshape(S, L, E)
    k = np.asarray(keys, dtype=np.float32).reshape(S, L, E)
    v = np.asarray(values, dtype=np.float32).reshape(S, L, E)
    sg = np.asarray(sigma, dtype=np.float32).reshape(S, L)

    in_maps = []
    for c in range(N_CORES):
        sl = slice(c * NSLICE, (c + 1) * NSLICE)
        in_maps.append(prep_core_inputs(q[sl], k[sl], v[sl], sg[sl]))

    res = bass_utils.run_bass_kernel_spmd(nc, in_maps, core_ids=list(range(N_CORES)))

    def gather(name, shape):
        return np.concatenate(
            [res.results[c][name] for c in range(N_CORES)], axis=0
        ).reshape(shape)

    V = gather("v_out", (B, C, H, L, E))
    series = gather("series_out", (B, C, H, L, L))
    prior = gather("prior_out", (B, C, H, L, L))
    sigma_b = gather("sigma_out", (B, C, H, L, L))
    return V, series, prior, sigma_b
